# revision 1
# baseline (speedup 1.0000x reference)
"""Trainium2 Bass kernel for nn_Encoder_8229157339656 (transformer encoder block).

Problem: B=2, S=2048, D=1024, H=16 heads, DH=64, F=4096, fp32 I/O.
  out = x + FFN(LN2(x + Attn(LN1(x))))   (pre-LN encoder block, eval mode)

Sharding (8 cores, zero cross-core communication):
  core c handles batch b=c//4, query rows q0=(c%4)*512 .. q0+512.
  Each core recomputes K/V for its full 2048-token batch locally (redundant
  across the 4 cores of a batch) — trades ~6.4 GFLOP/core of recompute for
  no collectives.

Numerics: all matmuls in bf16 with fp32 PSUM accumulation (measured e2e
rel err vs fp32 reference ~6e-4); LN stats, softmax denominators, and
residual adds in fp32. LN affine (g,b) and all linear biases are folded
into the weights host-side:
  h = (x-m)*rstd;  W' = diag(g)W;  b' = b_lin + b_ln @ W
  q,k biases applied per-partition on the PSUM->SBUF copy (k-dim on
  partitions in the qT/kT layout); v bias commutes through softmax
  (rows sum to 1) into the Wo bias; Wo/W2 biases added via a
  ones-block matmul against a host-replicated bias/128 block.
Softmax skips max-subtraction (scores are prescaled by 1/8 via Wq; range
is a few units, exp cannot overflow) and normalization is folded in after
the PV matmul using the ones-column-in-v trick to get row sums.

Device layouts (P=partition dim):
  hT/h2T   [128p=d%128, kd, tokens]   (d on partitions, via DMA transpose)
  qT/kT    [128p=2 heads x 64, pair, tokens]
  v        [128p=tokens, t-tile, head, 65]  (col 64 = ones, for denominators)
  eT       [128p=kv-tokens, t-tile, 512 q-tokens]
  oT       [128p=d%128, kd, 512]      (feeds Wo matmul directly)
  g1T      [128p=f%128, ft, 512]      (gelu output, feeds W2 directly)
"""

import numpy as np
import ml_dtypes
from contextlib import ExitStack

import concourse.bass as bass
import concourse.bacc as bacc
import concourse.tile as tile
import concourse.mybir as mybir
from concourse.bass_utils import run_bass_kernel_spmd
from concourse import masks

P = 128
D = 1024
F = 4096
H = 16
DH = 64
B = 2
S = 2048
SQ = 512          # query tokens per core
ND = D // P       # 8 d-tiles
NF = F // P       # 32 f-tiles
NT = S // P       # 16 kv token tiles
NQ = SQ // P      # 4 q token tiles
NCH = S // 512    # 4 kv 512-chunks
NPAIR = H // 2    # 8 head pairs
EPS = 1e-5
BF16 = mybir.dt.bfloat16
F32 = mybir.dt.float32
AF = mybir.ActivationFunctionType
ALU = mybir.AluOpType
NCORES = 8


def _layernorm_group(nc, stats_pool, xs, hs, eps_tile):
    """LN (no affine) of up to 8 [128, D] f32 APs -> bf16 APs.

    One DVE->ACT->DVE round trip per GROUP (sqrt+recip batched) so the DVE
    stream doesn't stall on the cross-engine hop per tile.
    """
    n = len(xs)
    mvG = stats_pool.tile([P, 8, 2], F32, name="bn_mv", tag="bn_mv")
    for j, x in enumerate(xs):
        st = stats_pool.tile([P, 2, 6], F32, name="bn_stats", tag="bn_stats")
        nc.vector.bn_stats(out=st[:, 0, :], in_=x[:, 0:512])
        nc.vector.bn_stats(out=st[:, 1, :], in_=x[:, 512:1024])
        nc.vector.bn_aggr(out=mvG[:, j, :], in_=st[:])
    # var col -> rstd = 1/sqrt(var+eps), batched over the group
    nc.scalar.activation(mvG[:, 0:n, 1], mvG[:, 0:n, 1], AF.Sqrt,
                         bias=eps_tile[:], scale=1.0)
    nc.vector.reciprocal(mvG[:, 0:n, 1], mvG[:, 0:n, 1])
    for j, (x, h) in enumerate(zip(xs, hs)):
        nc.vector.tensor_scalar(
            out=h[:], in0=x[:],
            scalar1=mvG[:, j, 0:1], scalar2=mvG[:, j, 1:2],
            op0=ALU.subtract, op1=ALU.mult,
        )


def _build_program():
    nc = bacc.Bacc("TRN2", target_bir_lowering=False, debug=False,
                   num_devices=NCORES)

    xb = nc.dram_tensor("xb", [S, D], F32, kind="ExternalInput").ap()
    xq = nc.dram_tensor("xq", [SQ, D], F32, kind="ExternalInput").ap()
    wq = nc.dram_tensor("wq", [D, D], BF16, kind="ExternalInput").ap()
    wk = nc.dram_tensor("wk", [D, D], BF16, kind="ExternalInput").ap()
    wv = nc.dram_tensor("wv", [D, D], BF16, kind="ExternalInput").ap()
    bq2 = nc.dram_tensor("bq2", [P, NPAIR], F32, kind="ExternalInput").ap()
    bk2 = nc.dram_tensor("bk2", [P, NPAIR], F32, kind="ExternalInput").ap()
    wo = nc.dram_tensor("wo", [D, D], BF16, kind="ExternalInput").ap()
    wob = nc.dram_tensor("wob", [P, D], BF16, kind="ExternalInput").ap()
    w1t = nc.dram_tensor("w1t", [NF, P, ND, P], BF16, kind="ExternalInput").ap()
    b1t = nc.dram_tensor("b1t", [P, NF], F32, kind="ExternalInput").ap()
    w2 = nc.dram_tensor("w2", [F, D], BF16, kind="ExternalInput").ap()
    w2b = nc.dram_tensor("w2b", [P, D], BF16, kind="ExternalInput").ap()
    out = nc.dram_tensor("out", [SQ, D], F32, kind="ExternalOutput").ap()

    def dram_tiled(ap, nt):
        # view [R, C] DRAM as [128p, nt, C] with p the fast row index
        return ap.rearrange("(t p) c -> p t c", p=P, t=nt)

    with tile.TileContext(nc) as tc:
        # Pools are managed with explicit alloc/release (two LIFO stacks,
        # side="left"/"right") so lifetimes can interleave across phases.
        const = tc.alloc_tile_pool(name="const", bufs=1)
        stats = tc.alloc_tile_pool(name="stats", bufs=8)
        lnp = tc.alloc_tile_pool(name="ln", bufs=5)
        # One PSUM scope for the whole kernel; tags reused across phases.
        # Banks: proj 2x1 + score 2x2 + attn 2x1 = 8.
        psum = tc.alloc_tile_pool(name="psum", bufs=2, space="PSUM")

        eps_tile = const.tile([P, 1], F32, name="eps", tag="eps")
        nc.vector.memset(eps_tile[:], EPS)
        ones_blk = const.tile([P, P], BF16, name="ones_blk", tag="ones_blk")
        nc.vector.memset(ones_blk[:], 1.0)
        bq2_sb = const.tile([P, NPAIR], F32, name="bq2_sb", tag="bq2_sb")
        nc.sync.dma_start(bq2_sb[:], bq2[:])
        bk2_sb = const.tile([P, NPAIR], F32, name="bk2_sb", tag="bk2_sb")
        nc.sync.dma_start(bk2_sb[:], bk2[:])
        b1t_sb = const.tile([P, NF], F32, name="b1t_sb", tag="b1t_sb")
        nc.sync.dma_start(b1t_sb[:], b1t[:])

        xq_t = dram_tiled(xq, NQ)
        xb_t = dram_tiled(xb, NT)
        out_t = dram_tiled(out, NQ)

        xq_pool = tc.alloc_tile_pool(name="xqp", bufs=1)
        xq_sb = xq_pool.tile([P, NQ, D], F32, name="xq_sb", tag="xq_sb")
        for i in range(NQ):
            nc.gpsimd.dma_start(xq_sb[:, i, :], xq_t[:, i, :])

        attn_pool = tc.alloc_tile_pool(name="attnp", bufs=1)
        qT = attn_pool.tile([P, NPAIR, SQ], BF16, name="qT", tag="qT")
        kT = attn_pool.tile([P, NPAIR, S], BF16, name="kT", tag="kT")
        vS = attn_pool.tile([P, NT, H, DH + 1], BF16, name="vS", tag="vS")

        # ========= Phases A+B: LN1 + projections ==========================
        # All 20 LN tiles stream on DVE with no PE coupling (PSUM copy-backs
        # live on ACT); hbT holds the full transposed batch so projections
        # never wait on a chunk rotation.
        hqT_pool = tc.alloc_tile_pool(name="hqTp", bufs=1)
        wpool = tc.alloc_tile_pool(name="wqkv", bufs=2)
        hbT_pool = tc.alloc_tile_pool(name="hbTp", bufs=1)

        ident = const.tile([P, P], BF16, name="ident", tag="ident")
        masks.make_identity(nc, ident[:])

        def pe_transpose_tile(h_tile, dest, col):
            # 8 PE transposes through PSUM (4 per bank), DVE copyback.
            # PE is in-order, so these rotate proj slots in program order.
            for half in range(2):
                ps_t = psum.tile([P, 4, P], BF16, name="tr", tag="proj")
                for k4 in range(4):
                    kd = half * 4 + k4
                    nc.tensor.transpose(ps_t[:, k4, :],
                                        h_tile[:, kd * P:(kd + 1) * P],
                                        ident[:])
                nc.vector.tensor_copy(
                    dest[:, half * 4:(half + 1) * 4, col:col + P], ps_t[:])

        wq_sb = wpool.tile([P, ND, D], BF16, name="wq_sb", tag="wmat")
        nc.gpsimd.dma_start(wq_sb[:], dram_tiled(wq, ND)[:])
        wk_sb = wpool.tile([P, ND, D], BF16, name="wk_sb", tag="wmat")
        nc.gpsimd.dma_start(wk_sb[:], dram_tiled(wk, ND)[:])

        hqT = hqT_pool.tile([P, ND, SQ], BF16, name="hqT", tag="hqT")
        hq_tiles = [lnp.tile([P, D], BF16, name="h_tile", tag="h_tile", bufs=10)
                    for _ in range(NQ)]
        _layernorm_group(nc, stats, [xq_sb[:, i, :] for i in range(NQ)],
                         hq_tiles, eps_tile)
        for i in range(NQ):
            pe_transpose_tile(hq_tiles[i], hqT, i * P)

        for p in range(NPAIR):
            ps = psum.tile([P, 512], F32, name="proj", tag="proj")
            for kd in range(ND):
                nc.tensor.matmul(ps[:], wq_sb[:, kd, p * P:(p + 1) * P],
                                 hqT[:, kd, :],
                                 start=(kd == 0), stop=(kd == ND - 1))
            nc.scalar.activation(qT[:, p, :], ps[:], AF.Identity,
                                 bias=bq2_sb[:, p:p + 1], scale=1.0)

        # wv reuses wq's slot (tag wmat) once qT is done
        wv_sb = wpool.tile([P, ND, D], BF16, name="wv_sb", tag="wmat")
        nc.gpsimd.dma_start(wv_sb[:], dram_tiled(wv, ND)[:])

        # ones cols for softmax denominators
        nc.vector.memset(vS[:, :, :, DH], 1.0)

        hbT = hbT_pool.tile([P, ND, S], BF16, name="hbT", tag="hbT")
        for c in range(NCH):
            # LN group for this 512-token chunk, then transposes, then
            # this chunk's kT / v projections (keeps PE fed chunk by chunk)
            xb_tiles = []
            for j in range(4):
                xb_i = lnp.tile([P, D], F32, name="xb_tile", tag="xb_tile",
                                bufs=5)
                nc.sync.dma_start(xb_i[:], xb_t[:, c * 4 + j, :])
                xb_tiles.append(xb_i)
            hb_tiles = [lnp.tile([P, D], BF16, name="h_tile", tag="h_tile",
                                 bufs=10) for _ in range(4)]
            _layernorm_group(nc, stats, xb_tiles, hb_tiles, eps_tile)
            for j in range(4):
                pe_transpose_tile(hb_tiles[j], hbT, (c * 4 + j) * P)

            for p in range(NPAIR):
                ps = psum.tile([P, 512], F32, name="proj", tag="proj")
                for kd in range(ND):
                    nc.tensor.matmul(ps[:], wk_sb[:, kd, p * P:(p + 1) * P],
                                     hbT[:, kd, c * 512:(c + 1) * 512],
                                     start=(kd == 0), stop=(kd == ND - 1))
                nc.scalar.activation(kT[:, p, c * 512:(c + 1) * 512],
                                     ps[:], AF.Identity,
                                     bias=bk2_sb[:, p:p + 1], scale=1.0)

            for j in range(4):
                t = c * 4 + j
                for ch in range(2):
                    ps = psum.tile([P, 512], F32, name="proj", tag="proj")
                    for kd in range(ND):
                        nc.tensor.matmul(ps[:], hbT[:, kd, t * P:(t + 1) * P],
                                         wv_sb[:, kd,
                                               ch * 512:(ch + 1) * 512],
                                         start=(kd == 0), stop=(kd == ND - 1))
                    # one strided copy: 8 heads' 64-col blocks into the
                    # 65-stride v layout
                    nc.scalar.activation(
                        vS[:, t, ch * 8:(ch + 1) * 8, 0:DH],
                        ps[:].rearrange("p (h k) -> p h k", k=DH), AF.Copy)

        hbT_pool.release()
        wpool.release()
        hqT_pool.release()

        # ========= Phase C: attention =====================================
        x2_pool = tc.alloc_tile_pool(name="x2p", bufs=1, side="right")
        x2 = x2_pool.tile([P, NQ, D], F32, name="x2", tag="x2")
        oT_pool = tc.alloc_tile_pool(name="oTp", bufs=1, side="right")
        oT = oT_pool.tile([P, ND, SQ], BF16, name="oT", tag="oT")

        eT_pool = tc.alloc_tile_pool(name="eTp", bufs=2)
        rec_pool = tc.alloc_tile_pool(name="recp", bufs=2)

        for h in range(H):
            p = h // 2
            off = (h % 2) * DH
            eT = eT_pool.tile([P, NT, SQ], BF16, name="eT", tag="eT")
            # two t-tiles per PSUM tile -> one exp per pair
            for tp in range(NT // 2):
                ps_s = psum.tile([P, 2, 512], F32, name="score", tag="score")
                for u in range(2):
                    t = 2 * tp + u
                    nc.tensor.matmul(
                        ps_s[:, u, :],
                        kT[off:off + DH, p, t * P:(t + 1) * P],
                        qT[off:off + DH, p, :],
                        start=True, stop=True)
                nc.scalar.activation(eT[:, 2 * tp:2 * tp + 2, :],
                                     ps_s[:], AF.Exp)
            ps_o = psum.tile([P, 512], F32, name="attn_ps", tag="attn_ps")
            for t in range(NT):
                nc.tensor.matmul(ps_o[0:DH + 1, :], vS[:, t, h, :],
                                 eT[:, t, :],
                                 start=(t == 0), stop=(t == NT - 1))
            recip = rec_pool.tile([1, SQ], F32, name="recip", tag="recip")
            nc.vector.reciprocal(recip[:], ps_o[DH:DH + 1, :])
            rb = rec_pool.tile([DH, SQ], F32, name="rb", tag="rb")
            nc.gpsimd.partition_broadcast(rb[:], recip[:])
            nc.vector.tensor_tensor(out=oT[off:off + DH, p, :],
                                    in0=ps_o[0:DH, :], in1=rb[:], op=ALU.mult)

        rec_pool.release()
        eT_pool.release()
        attn_pool.release()

        # W2 preload starts as soon as the attention tensors free up
        # (right-side stack so its lifetime can span D..G)
        w2_pool = tc.alloc_tile_pool(name="w2p", bufs=1, side="right")
        w2_sb = w2_pool.tile([P, NF, D], BF16, name="w2_sb", tag="w2_sb")
        # gpsimd (SWDGE) queues so this 8MB preload doesn't block the
        # w1c streaming loads on the sync HWDGE queues
        nc.gpsimd.dma_start(w2_sb[:], dram_tiled(w2, NF)[:])
        w2b_sb = w2_pool.tile([P, D], BF16, name="w2b_sb", tag="w2b_sb")
        nc.sync.dma_start(w2b_sb[:], w2b[:])

        # ========= Phase D: x2 = xq + oT @ Wo + bo ========================
        wod_pool = tc.alloc_tile_pool(name="wodp", bufs=1)
        wo_sb = wod_pool.tile([P, ND, D], BF16, name="wo_sb", tag="wo_sb")
        nc.gpsimd.dma_start(wo_sb[:], dram_tiled(wo, ND)[:])
        wob_sb = wod_pool.tile([P, D], BF16, name="wob_sb", tag="wob_sb")
        nc.sync.dma_start(wob_sb[:], wob[:])

        for st in range(NQ):
            for ch in range(2):
                ps_x = psum.tile([P, 512], F32, name="xps", tag="proj")
                for kd in range(ND):
                    nc.tensor.matmul(ps_x[:], oT[:, kd, st * P:(st + 1) * P],
                                     wo_sb[:, kd, ch * 512:(ch + 1) * 512],
                                     start=(kd == 0), stop=False)
                nc.tensor.matmul(ps_x[:], ones_blk[:],
                                 wob_sb[:, ch * 512:(ch + 1) * 512],
                                 start=False, stop=True)
                nc.vector.tensor_tensor(
                    out=x2[:, st, ch * 512:(ch + 1) * 512],
                    in0=ps_x[:], in1=xq_sb[:, st, ch * 512:(ch + 1) * 512],
                    op=ALU.add)

        wod_pool.release()
        xq_pool.release()

        # ========= Phases E..G: LN2 + FFN =================================
        h2T_pool = tc.alloc_tile_pool(name="h2Tp", bufs=1)
        w1_pool = tc.alloc_tile_pool(name="w1p", bufs=3)
        g1_pool = tc.alloc_tile_pool(name="g1p", bufs=1)
        out_pool = tc.alloc_tile_pool(name="outp", bufs=2)

        h2T = h2T_pool.tile([P, ND, SQ], BF16, name="h2T", tag="h2T")
        h2_tiles = [lnp.tile([P, D], BF16, name="h_tile", tag="h_tile", bufs=10)
                    for _ in range(NQ)]
        _layernorm_group(nc, stats, [x2[:, st, :] for st in range(NQ)],
                         h2_tiles, eps_tile)
        for st in range(NQ):
            pe_transpose_tile(h2_tiles[st], h2T, st * P)

        g1T = g1_pool.tile([P, NF, SQ], BF16, name="g1T", tag="g1T")
        for ft in range(NF):
            w1c = w1_pool.tile([P, ND, P], BF16, name="w1c", tag="w1c")
            nc.sync.dma_start(w1c[:], w1t[ft])
            ps_g = psum.tile([P, 512], F32, name="gps", tag="score")
            for kd in range(ND):
                nc.tensor.matmul(ps_g[:], w1c[:, kd, :], h2T[:, kd, :],
                                 start=(kd == 0), stop=(kd == ND - 1))
            nc.scalar.activation(g1T[:, ft, :], ps_g[:], AF.Gelu,
                                 bias=b1t_sb[:, ft:ft + 1], scale=1.0)

        for st in range(NQ):
            res_t = out_pool.tile([P, D], F32, name="res_t", tag="res_t")
            for ch in range(2):
                ps_f = psum.tile([P, 512], F32, name="fps", tag="attn_ps")
                for ft in range(NF):
                    nc.tensor.matmul(
                        ps_f[:], g1T[:, ft, st * P:(st + 1) * P],
                        w2_sb[:, ft, ch * 512:(ch + 1) * 512],
                        start=(ft == 0), stop=False)
                nc.tensor.matmul(ps_f[:], ones_blk[:],
                                 w2b_sb[:, ch * 512:(ch + 1) * 512],
                                 start=False, stop=True)
                nc.vector.tensor_tensor(
                    out=res_t[:, ch * 512:(ch + 1) * 512],
                    in0=ps_f[:], in1=x2[:, st, ch * 512:(ch + 1) * 512],
                    op=ALU.add)
            nc.sync.dma_start(out_t[:, st, :], res_t[:])

        out_pool.release()
        g1_pool.release()
        w1_pool.release()
        h2T_pool.release()
        w2_pool.release()
        oT_pool.release()
        x2_pool.release()
        psum.release()
        lnp.release()
        stats.release()
        const.release()

    nc.compile()
    return nc


def _prep_inputs(x, ln1_g, ln1_b, Wq, bq, Wk, bk, Wv, bv, Wo, bo,
                 ln2_g, ln2_b, W1, b1, W2, b2):
    """Host-side folding: returns per-core in_maps (list of 8 dicts)."""
    bf = ml_dtypes.bfloat16
    f32 = np.float32
    x = np.asarray(x, f32)
    ln1_g = np.asarray(ln1_g, f32); ln1_b = np.asarray(ln1_b, f32)
    ln2_g = np.asarray(ln2_g, f32); ln2_b = np.asarray(ln2_b, f32)
    Wqf = np.asarray(Wq, f32).transpose(1, 0, 2).reshape(D, D)
    Wkf = np.asarray(Wk, f32).transpose(1, 0, 2).reshape(D, D)
    Wvf = np.asarray(Wv, f32).transpose(1, 0, 2).reshape(D, D)
    bqf = np.asarray(bq, f32).reshape(D)
    bkf = np.asarray(bk, f32).reshape(D)
    bvf = np.asarray(bv, f32).reshape(D)
    Wo = np.asarray(Wo, f32); bo = np.asarray(bo, f32)
    W1 = np.asarray(W1, f32); b1 = np.asarray(b1, f32)
    W2 = np.asarray(W2, f32); b2 = np.asarray(b2, f32)

    isq = 1.0 / np.sqrt(DH)
    wq_eff = ((ln1_g[:, None] * Wqf) * isq).astype(bf)
    bq_eff = ((bqf + ln1_b @ Wqf) * isq).astype(f32)
    wk_eff = (ln1_g[:, None] * Wkf).astype(bf)
    bk_eff = (bkf + ln1_b @ Wkf).astype(f32)
    wv_eff = (ln1_g[:, None] * Wvf).astype(bf)
    bv_eff = bvf + ln1_b @ Wvf
    bo_eff = bo + bv_eff @ Wo
    wob = np.broadcast_to(bo_eff / P, (P, D)).astype(bf)
    w1_eff = (ln2_g[:, None] * W1).astype(bf)
    b1_eff = (b1 + ln2_b @ W1).astype(f32)
    w2b = np.broadcast_to(b2 / P, (P, D)).astype(bf)

    shared = {
        "wq": np.ascontiguousarray(wq_eff),
        "wk": np.ascontiguousarray(wk_eff),
        "wv": np.ascontiguousarray(wv_eff),
        "bq2": np.ascontiguousarray(bq_eff.reshape(NPAIR, P).T),
        "bk2": np.ascontiguousarray(bk_eff.reshape(NPAIR, P).T),
        "wo": np.ascontiguousarray(Wo.astype(bf)),
        "wob": np.ascontiguousarray(wob),
        "w1t": np.ascontiguousarray(
            w1_eff.reshape(ND, P, NF, P).transpose(2, 1, 0, 3)),
        "b1t": np.ascontiguousarray(b1_eff.reshape(NF, P).T),
        "w2": np.ascontiguousarray(W2.astype(bf)),
        "w2b": np.ascontiguousarray(w2b),
    }
    in_maps = []
    for c in range(NCORES):
        b = c // 4
        q0 = (c % 4) * SQ
        m = dict(shared)
        m["xb"] = np.ascontiguousarray(x[b])
        m["xq"] = np.ascontiguousarray(x[b, q0:q0 + SQ])
        in_maps.append(m)
    return in_maps


_NC_CACHE = None


def _get_program():
    global _NC_CACHE
    if _NC_CACHE is None:
        _NC_CACHE = _build_program()
    return _NC_CACHE


def kernel(**inputs) -> np.ndarray:
    nc = _get_program()
    in_maps = _prep_inputs(**inputs)
    res = run_bass_kernel_spmd(nc, in_maps, list(range(NCORES)))
    out = np.empty((B, S, D), np.float32)
    for c in range(NCORES):
        b = c // 4
        q0 = (c % 4) * SQ
        out[b, q0:q0 + SQ] = res.results[c]["out"]
    return out


if __name__ == "__main__":
    import reference as R
    inp = {k: np.asarray(v) for k, v in R.setup_inputs().items()}
    exp = np.asarray(R.reference(**inp))
    act = kernel(**inp)
    err = np.abs(act - exp)
    print("absmax err:", err.max(), "rel:", err.max() / np.abs(exp).max())

